# revision 16
# baseline (speedup 1.0000x reference)
"""Bass/Tile TRN2 kernel for nn_MessageAggregation.

Computes: s = sum_n e2[n]; out = leaky_relu((e1+s) @ W1.T + (e1*s) @ W2.T)

Sharding: data-parallel over batch B=8192 across 8 NeuronCores (1024 rows
per core); W1/W2 replicated.

Per-core layout: SBUF [128 partitions, 1024 free]; partition p holds batch
rows 8p..8p+7 (4 KB contiguous per partition per DMA descriptor). The kernel
is DMA-bound (~32 MB of all_embeddings2 per core; ~403 GB/s with 2 MB
loads and deep buffering so consecutive DMAs overlap on the ring), so the
n-reduction is split to hide under the stream, per 4-slice load:
  - 2 slices -> DVE tensor_add into a PSUM-resident accumulator (the mixed
    PSUM/SBUF operand path measured 1.23 us/slice vs 3.3 us SBUF-only)
  - 1 slice  -> TensorEngine via fp32 identity-matmul PSUM accumulation
    (exact; interleaved multi-region transpose-accumulate corrupts, so the
    transpose happens once in the tail instead)
  - 1 slice  -> GpSimd tensor_add (SBUF accumulator)
The final loads taper to 1 MB and drop GpSimd/PE so their partials fold
mid-stream (GpSimd's into the DVE accumulator on DVE; PE's staged to SBUF
by the Scalar engine). e1^T is pre-staged during the stream. Tail, per
128-column chunk: one fold add, one PE transpose produces the s^T chunk in
a rotating PSUM tile, then:
x1^T = e1^T + s^T and x2^T = e1^T * s^T on DVE (PSUM-mixed, no copies),
two matmuls (x^T stationary, W^T moving) -> PSUM [b, o], leaky relu as
max(x, 0.01x), per-chunk store.
"""

import sys

for _p in ("/opt/trn_rl_repo",):
    if _p not in sys.path:
        sys.path.insert(0, _p)

import numpy as np

import concourse.bacc as bacc
import concourse.mybir as mybir
import concourse.tile as tile
from concourse.masks import make_identity
from concourse.bass_utils import run_bass_kernel_spmd

B, N, D = 8192, 64, 128
M = 8  # cores
BL = B // M  # 1024 rows per core
R = BL // 128  # chunks per core (8)
F = BL  # free width of the [128, F] working layout
H = F // 2
G = 4  # n-slices per DMA load (2 MB per load)
NL = N // G
F32 = mybir.dt.float32
NEG_SLOPE = 0.01

def build(load_bufs: int = 8):
    nc = bacc.Bacc(
        "TRN2",
        target_bir_lowering=False,
        debug=False,
        enable_asserts=False,
        num_devices=M,
    )
    e1 = nc.dram_tensor("embedding1", [BL, D], F32, kind="ExternalInput").ap()
    e2 = nc.dram_tensor("all_embeddings2", [N, BL, D], F32, kind="ExternalInput").ap()
    w1 = nc.dram_tensor("W1", [D, D], F32, kind="ExternalInput").ap()
    w2 = nc.dram_tensor("W2", [D, D], F32, kind="ExternalInput").ap()
    out = nc.dram_tensor("out", [BL, D], F32, kind="ExternalOutput").ap()

    e1_r = e1.rearrange("(p r) d -> p (r d)", p=128)  # [128, 1024]
    out_r = out.rearrange("(p r) d -> p (r d)", p=128)
    e2_r = e2.rearrange("n (p r) d -> p n (r d)", p=128)  # [128, 64, 1024]

    # Load plan: 14 x 2MB loads then 4 x 1MB loads (tapered tail so the
    # final load's consumers lag as little as possible). Routing letters:
    # D -> DVE, P -> PE, G -> GpSimd. GpSimd's and PE's last slices land
    # before the final loads so their partials fold mid-stream.
    plan = [(4, "DDPG")] * 14 + [(2, "DG"), (2, "DP"), (2, "DP"), (2, "DD")]
    assert sum(g for g, _ in plan) == N

    with tile.TileContext(nc) as tc:
        with (
            tc.tile_pool(name="const", bufs=1) as cpool,
            tc.tile_pool(name="load", bufs=load_bufs) as lpool,
            tc.tile_pool(name="act", bufs=1) as apool,
            tc.tile_pool(name="xt", bufs=2) as xtpool,
            tc.tile_pool(name="sdve", bufs=1, space="PSUM") as sdpool,
            tc.tile_pool(name="stps", bufs=1, space="PSUM") as stpool,
            tc.tile_pool(name="trps", bufs=2, space="PSUM") as trpool,
            tc.tile_pool(name="ops", bufs=2, space="PSUM") as opool,
        ):
            ident = cpool.tile([128, 128], F32)
            make_identity(nc, ident[:])

            w1_sb = cpool.tile([128, 128], F32)
            nc.scalar.dma_start(out=w1_sb[:], in_=w1)
            w2_sb = cpool.tile([128, 128], F32)
            nc.scalar.dma_start(out=w2_sb[:], in_=w2)
            e1_sb = apool.tile([128, F], F32)
            nc.scalar.dma_start(out=e1_sb[:], in_=e1_r)

            # W.T in SBUF: moving operand of the output matmuls.
            w1t_ps = trpool.tile([128, 128], F32, tag="tr")
            nc.tensor.transpose(w1t_ps[:], w1_sb[:], ident[:])
            w1t = cpool.tile([128, 128], F32)
            nc.scalar.copy(out=w1t[:], in_=w1t_ps[:])
            w2t_ps = trpool.tile([128, 128], F32, tag="tr")
            nc.tensor.transpose(w2t_ps[:], w2_sb[:], ident[:])
            w2t = cpool.tile([128, 128], F32)
            nc.scalar.copy(out=w2t[:], in_=w2t_ps[:])

            # e1^T pre-stage: chunk j of e1 transposed -> e1t[:, j*128:(j+1)*128]
            e1t = apool.tile([128, F], F32)
            for j in range(R):
                sl = slice(j * 128, (j + 1) * 128)
                tp = trpool.tile([128, 128], F32, tag="tr")
                nc.tensor.transpose(tp[:], e1_sb[:, sl], ident[:])
                nc.scalar.copy(out=e1t[:, sl], in_=tp[:])

            # ---- stream ----
            s_dve = sdpool.tile([128, F], F32)  # PSUM accumulator (2 banks)
            s_pe = stpool.tile([128, F], F32)  # PE identity-mm accumulator (2 banks)
            s_gps = apool.tile([128, F], F32)  # SBUF accumulator
            s_pe_sb = apool.tile([128, F], F32)  # mid-stream ACT copy of s_pe
            n_pe = sum(r.count("P") for _, r in plan)
            n_gps = sum(r.count("G") for _, r in plan)
            seen = {"D": 0, "P": 0, "G": 0}
            base = 0
            for gl, routing in plan:
                t = lpool.tile([128, gl * F], F32, tag="load")
                nc.sync.dma_start(
                    out=t[:].rearrange("p (n f) -> p n f", n=gl),
                    in_=e2_r[:, base : base + gl, :],
                )
                for g in range(gl):
                    eng = routing[g]
                    seen[eng] += 1
                    sl = t[:, g * F : (g + 1) * F]
                    if eng == "D":
                        if seen["D"] == 1:
                            nc.vector.tensor_copy(out=s_dve[:], in_=sl)
                        else:
                            nc.vector.tensor_add(out=s_dve[:], in0=s_dve[:], in1=sl)
                    elif eng == "P":
                        for h in range(2):
                            nc.tensor.matmul(
                                s_pe[:, h * 512 : (h + 1) * 512],
                                lhsT=ident[:],
                                rhs=sl[:, h * 512 : (h + 1) * 512],
                                start=(seen["P"] == 1),
                                stop=(seen["P"] == n_pe),
                            )
                        if seen["P"] == n_pe:
                            # PE partial done: stage it to SBUF on idle ACT
                            for h in range(2):
                                hs = slice(h * H, (h + 1) * H)
                                nc.scalar.copy(out=s_pe_sb[:, hs], in_=s_pe[:, hs])
                    else:
                        if seen["G"] == 1:
                            nc.gpsimd.tensor_copy(out=s_gps[:], in_=sl)
                        else:
                            nc.gpsimd.tensor_add(out=s_gps[:], in0=s_gps[:], in1=sl)
                            if seen["G"] == n_gps:
                                # GpSimd partial done: fold it into s_dve now
                                nc.vector.tensor_add(
                                    out=s_dve[:], in0=s_dve[:], in1=s_gps[:]
                                )
                base += gl

            # ---- tail ----
            # fold per chunk: s = s_dve (PSUM, incl GpSimd part) + s_pe (SBUF)
            s_sb = apool.tile([128, F], F32)
            x1t = apool.tile([128, F], F32)
            x2t = apool.tile([128, F], F32)
            for j in range(R):
                sl = slice(j * 128, (j + 1) * 128)
                nc.vector.tensor_add(out=s_sb[:, sl], in0=s_pe_sb[:, sl], in1=s_dve[:, sl])
                # s^T chunk in a rotating PSUM tile, consumed directly by DVE
                tp = trpool.tile([128, 128], F32, tag="tr")
                nc.tensor.transpose(tp[:], s_sb[:, sl], ident[:])
                nc.vector.tensor_add(out=x1t[:, sl], in0=e1t[:, sl], in1=tp[:])
                nc.vector.tensor_mul(out=x2t[:, sl], in0=e1t[:, sl], in1=tp[:])

                o_ps = opool.tile([128, 128], F32, tag="o")
                nc.tensor.matmul(o_ps[:], lhsT=x1t[:, sl], rhs=w1t[:], start=True, stop=False)
                nc.tensor.matmul(o_ps[:], lhsT=x2t[:, sl], rhs=w2t[:], start=False, stop=True)

                # leaky_relu(x) = max(x, 0.01 x)
                neg = xtpool.tile([128, 128], F32, tag="neg")
                nc.scalar.mul(neg[:], o_ps[:], NEG_SLOPE)
                out_sb = xtpool.tile([128, 128], F32, tag="osb")
                nc.vector.tensor_max(out=out_sb[:], in0=o_ps[:], in1=neg[:])
                nc.scalar.dma_start(out=out_r[:, sl], in_=out_sb[:])

    nc.compile()
    return nc


_NC = None


def _get_nc():
    global _NC
    if _NC is None:
        _NC = build()
    return _NC


def _make_in_maps(inputs):
    e1 = np.asarray(inputs["embedding1"], dtype=np.float32)
    e2 = np.asarray(inputs["all_embeddings2"], dtype=np.float32)
    w1 = np.asarray(inputs["W1"], dtype=np.float32)
    w2 = np.asarray(inputs["W2"], dtype=np.float32)
    in_maps = []
    for k in range(M):
        sl = slice(k * BL, (k + 1) * BL)
        in_maps.append(
            {
                "embedding1": np.ascontiguousarray(e1[sl]),
                "all_embeddings2": np.ascontiguousarray(e2[:, sl, :]),
                "W1": w1,
                "W2": w2,
            }
        )
    return in_maps


def _run(inputs, trace=False, **kwargs):
    nc = _get_nc()
    res = run_bass_kernel_spmd(
        nc, _make_in_maps(inputs), core_ids=list(range(M)), trace=trace, **kwargs
    )
    full = np.concatenate([res.results[k]["out"] for k in range(M)], axis=0)
    return full, res


def kernel(**inputs):
    full, _ = _run(inputs)
    return full
